# revision 46
# baseline (speedup 1.0000x reference)
"""Fused multi-head attention block (QKV proj + per-head RMSNorm + RoPE +
softmax attention + output proj) on 8 Trainium2 NeuronCores.

Sharding: core c handles (batch b = c//2, head-group hg = c%2 of 8 heads).
Each core computes a partial output projection over its 8 heads; the host
sums the two partials per batch.

Optimizations over the straightforward schedule:
- q/k are produced directly transposed (d, t) by making the weight tile the
  matmul's stationary operand, which removes all PE transposes; RMS-norm
  sums become ones-matmuls over the partition dim, and rstd rows are
  broadcast down partitions by gpsimd.
- heads are processed in pairs so DVE/ACT ops are 1024 elements wide
  (per-op fixed overhead amortized).
- the softmax denominator is a wide strided DVE add-tree (16 -> 2) plus a
  2-step accumulating ones-matmul, instead of 8 ones-matmuls.
- reciprocals use the fast approximate DVE op (~18 bits, plenty for bf16).
- PV matmuls and output-projection matmuls are interleaved between score
  matmuls: exp on ACT is slower than the matmuls that fill each score
  bank, so the PE is given sp-independent work to chew on.

Self-contained: hardcodes B=4, T=2048, C=2048, H=16, D=128.
"""

import math
import sys
import types

import numpy as np
import ml_dtypes

import concourse.bass as bass
import concourse.bacc as bacc
import concourse.tile as tile
from concourse import mybir
from concourse.bass_utils import run_bass_kernel_spmd

BF16 = mybir.dt.bfloat16
F32 = mybir.dt.float32
NP_BF16 = ml_dtypes.bfloat16
AF = mybir.ActivationFunctionType
ALU = mybir.AluOpType

B, T, C, H, D = 4, 2048, 2048, 16, 128
HL = H // 2  # heads per core
EPS = 1e-6
NCORES = 8


def _bcast_mid(ap2d, n):
    """[P, F] AP -> [P, n, F] AP broadcast along a new middle dim (step 0)."""
    return bass.AP(
        tensor=ap2d.tensor,
        offset=ap2d.offset,
        ap=[ap2d.ap[0], [0, n], ap2d.ap[1]],
    )


def build(T_=T):
    """Build + compile the per-core Bass program (identical on all cores)."""
    nt = T_ // 128  # number of 128-row t-tiles
    ng = T_ // 512  # number of 512-col t-chunks
    ncl = C // 128  # contraction tiles over C

    nc = bacc.Bacc("TRN2", target_bir_lowering=False, debug=False, num_devices=NCORES)

    xt = nc.dram_tensor("xt", [C, T_], BF16, kind="ExternalInput")  # x[b].T
    # [g, c_part, c_tile, d]: g = 8 k-heads then 8 q-heads (k lands first);
    # per-g slice is 4KB/partition contiguous for fast DMA
    wqk = nc.dram_tensor("wqk", [16, 128, ncl, D], BF16, kind="ExternalInput")
    wv = nc.dram_tensor("wv", [C, HL * D], BF16, kind="ExternalInput")
    wp = nc.dram_tensor("wp", [HL, D, C], BF16, kind="ExternalInput")
    # rope tables transposed to (d, t); q tables carry g_q and 1/sqrt(D)
    cq = nc.dram_tensor("cq", [D, T_], F32, kind="ExternalInput")
    sq = nc.dram_tensor("sq", [D, T_], F32, kind="ExternalInput")
    ck = nc.dram_tensor("ck", [D, T_], F32, kind="ExternalInput")
    sk = nc.dram_tensor("sk", [D, T_], F32, kind="ExternalInput")
    out = nc.dram_tensor("out", [T_, C], F32, kind="ExternalOutput")

    with tile.TileContext(nc) as tc:
        with (
            tc.tile_pool(name="persist", bufs=1) as persist,
            tc.tile_pool(name="dram", bufs=1, space="DRAM") as dpool,
        ):
            ones_b = persist.tile([128, 1], BF16)
            nc.vector.memset(ones_b[:], 1.0)
            epsb = persist.tile([128, 1], F32)
            nc.vector.memset(epsb[:], EPS)

            KT = persist.tile([128, HL, T_], BF16)  # normalized k^T: (d, h, t)
            qt_dram = dpool.tile([HL, D, T_], BF16)  # q^T spill: (h, d, t)
            vs_dram = dpool.tile([128, nt, HL, D], BF16)  # v spill

            # ---------- phase 1: QKV proj + RMS norm + RoPE ----------
            with (
                tc.tile_pool(name="wqk_pool", bufs=1) as wqk_pool,
                tc.tile_pool(name="wv_pool", bufs=1) as wv_pool,
                tc.tile_pool(name="x_pool", bufs=2) as x_pool,
                tc.tile_pool(name="cs_pool", bufs=1) as cs_pool,
                tc.tile_pool(name="work", bufs=1) as work,
                tc.tile_pool(name="wk2", bufs=2) as wk2,
                tc.tile_pool(name="qk_ps", bufs=3, space="PSUM") as qk_ps,
                tc.tile_pool(name="s1_ps", bufs=2, space="PSUM") as s1_ps,
            ):
                def load_xp(ch):
                    xp = x_pool.tile([128, ncl, 512], BF16, tag="xp")
                    for cc in range(4):
                        nc.scalar.dma_start(
                            out=xp[:, cc * 4 : (cc + 1) * 4, :],
                            in_=xt[:].rearrange("(n p) t -> p n t", p=128)[
                                :, cc * 4 : (cc + 1) * 4, ch * 512 : (ch + 1) * 512
                            ],
                        )
                    return xp

                def load_tabs(ch):
                    tabs = {}
                    for nm, dram in (("cq", cq), ("sq", sq), ("ck", ck), ("sk", sk)):
                        tab = cs_pool.tile([128, 512], F32, tag=nm, name=nm)
                        nc.scalar.dma_start(
                            out=tab[:], in_=dram[:, ch * 512 : (ch + 1) * 512]
                        )
                        tabs[nm] = tab
                    return tabs

                def post_pair(ch, j, ps2, tabs):
                    """RMS-normalize + rope one head pair (k if j<4 else q)."""
                    is_k = j < 4
                    h0 = (j - (0 if is_k else 4)) * 2
                    # sum of squares per t (ones-matmul over the d partitions)
                    sqt2 = work.tile([128, 2, 512], BF16, tag="sqt2")
                    nc.scalar.activation(sqt2[:], ps2[:], AF.Square)
                    # rstd row = 1/sqrt(ss/D + eps), then broadcast down partitions
                    rb2 = wk2.tile([128, 2, 512], F32, tag="rb2")
                    for u in range(2):
                        ss = s1_ps.tile([1, 512], F32, tag="ss")
                        nc.tensor.matmul(
                            ss[:], lhsT=ones_b[:], rhs=sqt2[:, u, :],
                            start=True, stop=True,
                        )
                        sqb = work.tile([1, 512], F32, tag="sqb")
                        nc.scalar.activation(
                            sqb[:], ss[:], AF.Sqrt, bias=epsb[0:1, :], scale=1.0 / D
                        )
                        rr = work.tile([1, 512], F32, tag="rr")
                        nc.vector.reciprocal_approx_fast(rr[:], sqb[:])
                        nc.gpsimd.partition_broadcast(rb2[:, u, :], rr[:])
                    # rope (wide ops over the pair; sign folded into sin table)
                    ct, st = (tabs["ck"], tabs["sk"]) if is_k else (
                        tabs["cq"], tabs["sq"])
                    t3a = work.tile([128, 2, 512], F32, tag="t3a")
                    nc.vector.tensor_mul(t3a[:], ps2[:], _bcast_mid(ct[:], 2))
                    t3b = work.tile([128, 2, 512], F32, tag="t3b")
                    nc.vector.tensor_mul(
                        t3b[0:64, :, :], ps2[64:128, :, :],
                        _bcast_mid(st[0:64, :], 2),
                    )
                    nc.vector.tensor_mul(
                        t3b[64:128, :, :], ps2[0:64, :, :],
                        _bcast_mid(st[64:128, :], 2),
                    )
                    t3 = wk2.tile([128, 2, 512], F32, tag="t3")
                    nc.vector.tensor_add(t3[:], t3a[:], t3b[:])
                    if is_k:
                        nc.vector.tensor_mul(
                            KT[:, h0 : h0 + 2, ch * 512 : (ch + 1) * 512],
                            t3[:], rb2[:],
                        )
                    else:
                        qbf2 = work.tile([128, 2, 512], BF16, tag="qbf2")
                        nc.vector.tensor_mul(qbf2[:], t3[:], rb2[:])
                        nc.scalar.dma_start(
                            out=qt_dram[
                                h0 : h0 + 2, :, ch * 512 : (ch + 1) * 512
                            ].rearrange("h d t -> d h t"),
                            in_=qbf2[:],
                        )

                xp = load_xp(0)
                tabs = load_tabs(0)
                WQK = wqk_pool.tile([128, ncl, 16, D], BF16)
                for g in range(16):  # k heads 0-7 first, then q heads
                    # g2/g3 ride the scalar queue so the PE doesn't outrun
                    # the sync queue's weight stream in the first pairs
                    q = nc.scalar if g in (2, 3) else nc.sync
                    q.dma_start(out=WQK[:, :, g, :], in_=wqk[g])
                WV = wv_pool.tile([128, ncl, HL * D], BF16)
                for cc in range(4):
                    nc.sync.dma_start(
                        out=WV[:, cc * 4 : (cc + 1) * 4, :],
                        in_=wv[:].rearrange("(n p) f -> p n f", p=128)[
                            :, cc * 4 : (cc + 1) * 4, :
                        ],
                    )
                for ch in range(ng):
                    # post-processing lags one item so the PE never waits on
                    # the ACT/DVE chain of the work it just produced.  The
                    # last chunk runs k, v, q so the sync queue's v spills
                    # finish early and phase 2's prefetches start sooner.
                    if ch + 1 < ng:
                        items = [("p", j) for j in range(8)]
                        items[4:4] = []
                        items += [("v", ts) for ts in range(4)]
                    else:
                        items = (
                            [("p", j) for j in range(4)]
                            + [("v", ts) for ts in range(4)]
                            + [("p", j) for j in range(4, 8)]
                        )
                    prev = None
                    for nitem, (kind, j) in enumerate(items):
                        ps2 = qk_ps.tile([128, 2, 512], F32, tag="qkps")
                        if kind == "p":
                            for u in range(2):
                                g = 2 * j + u if j < 4 else 8 + (j - 4) * 2 + u
                                for c in range(ncl):
                                    nc.tensor.matmul(
                                        ps2[:, u, :],
                                        lhsT=WQK[:, c, g, :],
                                        rhs=xp[:, c, :],
                                        start=(c == 0),
                                        stop=(c == ncl - 1),
                                    )
                        else:
                            for u in range(2):
                                for c in range(ncl):
                                    nc.tensor.matmul(
                                        ps2[:, u, :],
                                        lhsT=xp[:, c, j * 128 : (j + 1) * 128],
                                        rhs=WV[:, c, u * 512 : (u + 1) * 512],
                                        start=(c == 0),
                                        stop=(c == ncl - 1),
                                    )
                        if prev is not None:
                            pk, pj, pps = prev
                            if pk == "p":
                                post_pair(ch, pj, pps, tabs)
                            else:
                                vst = work.tile([128, 1024], BF16, tag="vst")
                                nc.scalar.copy(vst[:], pps[:])
                                nc.sync.dma_start(
                                    out=vs_dram[:, ch * 4 + pj, :, :],
                                    in_=vst[:].rearrange("p (h d) -> p h d", h=HL),
                                )
                        prev = (kind, j, ps2)
                        if nitem == 3 and ch + 1 < ng:
                            xp_next = load_xp(ch + 1)
                            tabs_next = load_tabs(ch + 1)
                    pk, pj, pps = prev
                    if pk == "p":
                        post_pair(ch, pj, pps, tabs)
                    else:
                        vst = work.tile([128, 1024], BF16, tag="vst")
                        nc.scalar.copy(vst[:], pps[:])
                        nc.sync.dma_start(
                            out=vs_dram[:, ch * 4 + pj, :, :],
                            in_=vst[:].rearrange("p (h d) -> p h d", h=HL),
                        )
                    if ch + 1 < ng:
                        xp = xp_next
                        tabs = tabs_next

            # ---------- phase 2: attention + output projection ----------
            with (
                tc.tile_pool(name="vs_pool", bufs=1) as vs_pool,
                tc.tile_pool(name="wp_pool", bufs=1) as wp_pool,
                tc.tile_pool(name="qt_pool", bufs=2) as qt_pool,
                tc.tile_pool(name="pt_pool", bufs=3) as pt_pool,
                tc.tile_pool(name="tree_pool", bufs=1) as tree_pool,
                tc.tile_pool(name="y_pool", bufs=2) as y_pool,
                tc.tile_pool(name="o_pool", bufs=3) as o_pool,
                tc.tile_pool(name="r_pool", bufs=2) as r_pool,
                tc.tile_pool(name="rb_pool", bufs=2) as rb_pool,
                tc.tile_pool(name="sp_ps", bufs=2, space="PSUM") as sp_ps,
                tc.tile_pool(name="acc_ps", bufs=4, space="PSUM") as acc_ps,
            ):
                def load_qtc(ch, queue=None):
                    qtc = qt_pool.tile([128, HL, 512], BF16, tag="qtc")
                    (queue or nc.scalar).dma_start(
                        out=qtc[:],
                        in_=qt_dram[:, :, ch * 512 : (ch + 1) * 512].rearrange(
                            "h d t -> d h t"
                        ),
                    )
                    return qtc

                # the sync queue is idle after the early weight loads, and this
                # load only waits on chunk 0's q spill: it runs mid-phase-1
                qtc_next = load_qtc(0, queue=nc.sync)
                Vs = vs_pool.tile([128, nt, HL, D], BF16)
                nc.sync.dma_start(out=Vs[:], in_=vs_dram[:])
                WPT = wp_pool.tile([128, HL, C], BF16)
                nc.sync.dma_start(out=WPT[:], in_=wp[:].rearrange("h d o -> d h o"))

                def emit_scores(qtc, h, PT, t_old, den_sg=1):
                    """S^T = K^T.T @ q^T; exp into PT (2-tile ACT ops).

                    The task from two iterations ago has its PV matmuls
                    interleaved between score matmuls: exp is slower than the
                    matmul that fills each score bank, so the PE needs
                    sp-independent work in between.
                    """
                    yp = None
                    if t_old is not None:
                        yp = acc_ps.tile([128, 512], F32, tag="acc")
                    for sg in range(nt // 2):
                        sp = sp_ps.tile([128, 2, 512], F32, tag="sp")
                        for u in range(2):
                            i = 2 * sg + u
                            nc.tensor.matmul(
                                sp[:, u, :],
                                lhsT=KT[:, h, i * 128 : (i + 1) * 128],
                                rhs=qtc[:, h, :],
                                start=True,
                                stop=True,
                            )
                        nc.scalar.activation(
                            PT[:, 2 * sg : 2 * sg + 2, :], sp[:], AF.Exp
                        )
                        if t_old is not None:
                            if sg == den_sg:
                                emit_denominator(t_old)
                            for u in range(2):
                                i = 2 * sg + u
                                nc.tensor.matmul(
                                    yp[:],
                                    lhsT=Vs[:, i, t_old["h"], :],
                                    rhs=t_old["PT"][:, i, :],
                                    start=(i == 0),
                                    stop=(i == nt - 1),
                                )
                        proj_step()
                    return yp

                def emit_tree(t):
                    """Pairwise add-tree 16 -> 2 as three wide strided DVE ops."""
                    PT = t["PT"]
                    ta = tree_pool.tile([128, 8, 512], BF16, tag="ta")
                    nc.vector.tensor_add(ta[:], PT[:, 0:16:2, :], PT[:, 1:16:2, :])
                    tb = tree_pool.tile([128, 4, 512], BF16, tag="tb")
                    nc.vector.tensor_add(tb[:], ta[:, 0:8:2, :], ta[:, 1:8:2, :])
                    tc2 = tree_pool.tile([128, 2, 512], BF16, tag="tc2")
                    nc.vector.tensor_add(tc2[:], tb[:, 0:4:2, :], tb[:, 1:4:2, :])
                    t["td"] = tc2

                def emit_denominator(t):
                    """Denominator ones-matmul + approx reciprocal + broadcast."""
                    ss = acc_ps.tile([1, 512], F32, tag="acc")
                    td = t["td"]
                    for j in range(2):
                        nc.tensor.matmul(ss[:], lhsT=ones_b[:], rhs=td[:, j, :],
                                         start=(j == 0), stop=(j == 1))
                    rinv = r_pool.tile([1, 512], F32, tag="rinv")
                    nc.vector.reciprocal_approx_fast(rinv[:], ss[:])
                    rbs = rb_pool.tile([128, 512], F32, tag="rbs")
                    nc.gpsimd.partition_broadcast(rbs[:], rinv[:])
                    t["rbs"] = rbs

                # output projection runs as a work queue: two matmuls are
                # drained per score group, absorbing the projection into the
                # exp-paced stretches where the PE otherwise has spare cycles
                proj_units = []  # (Ysb, trow, ot) per op-tile
                proj_cur = {}

                def proj_step():
                    if not proj_cur:
                        if not proj_units:
                            return
                        Ysb, trow, ot = proj_units.pop(0)
                        op = acc_ps.tile([128, 512], F32, tag="acc", name="op")
                        proj_cur.update(Ysb=Ysb, trow=trow, ot=ot, h=0, op=op)
                    Ysb, trow, ot = proj_cur["Ysb"], proj_cur["trow"], proj_cur["ot"]
                    op, h = proj_cur["op"], proj_cur["h"]
                    ts = trow % 4
                    for hh in (h, h + 1):
                        nc.tensor.matmul(
                            op[:],
                            lhsT=Ysb[:, hh, ts * 128 : (ts + 1) * 128],
                            rhs=WPT[:, hh, ot * 512 : (ot + 1) * 512],
                            start=(hh == 0),
                            stop=(hh == HL - 1),
                        )
                    proj_cur["h"] = h + 2
                    if proj_cur["h"] == HL:
                        ost = o_pool.tile([128, 512], F32, tag="ost")
                        nc.vector.tensor_copy(ost[:], op[:])
                        nc.sync.dma_start(
                            out=out[:].rearrange("(n p) o -> n p o", p=128)[
                                trow
                            ][:, ot * 512 : (ot + 1) * 512],
                            in_=ost[:],
                        )
                        proj_cur.clear()

                def emit_proj(ch, Ysb):
                    """Queue the output projection for one finished chunk."""
                    for ts in range(4):
                        for ot in range(C // 512):
                            proj_units.append((Ysb, ch * 4 + ts, ot))

                pending = []  # task dicts; PV interleaves two iterations later

                def finish(t, yp):
                    nc.vector.tensor_mul(t["Ysb"][:, t["h"], :], yp[:], t["rbs"][:])
                    if t["h"] == HL - 1:
                        emit_proj(t["ch"], t["Ysb"])

                for ch in range(ng):
                    qtc = qtc_next
                    Ysb = y_pool.tile([128, HL, 512], BF16, tag="y")
                    for h in range(HL):
                        if ch == 0 and h == 0:
                            # warmup: run h0+h1 scores in one fused slot so
                            # the PE has 32 matmuls against the same
                            # exp-paced ACT stretch instead of idling
                            PT0 = pt_pool.tile([128, nt, 512], BF16, tag="pt")
                            PT1 = pt_pool.tile([128, nt, 512], BF16, tag="pt")
                            for sg in range(nt // 2):
                                for hh, PTx in ((0, PT0), (1, PT1)):
                                    sp = sp_ps.tile([128, 2, 512], F32, tag="sp")
                                    for u in range(2):
                                        i = 2 * sg + u
                                        nc.tensor.matmul(
                                            sp[:, u, :],
                                            lhsT=KT[:, hh, i * 128 : (i + 1) * 128],
                                            rhs=qtc[:, hh, :],
                                            start=True,
                                            stop=True,
                                        )
                                    nc.scalar.activation(
                                        PTx[:, 2 * sg : 2 * sg + 2, :], sp[:],
                                        AF.Exp,
                                    )
                            qtc_next = load_qtc(ch + 1)
                            pending.append({"ch": 0, "h": 0, "PT": PT0,
                                            "td": None, "Ysb": Ysb})
                            emit_tree(pending[-1])
                            pending.append({"ch": 0, "h": 1, "PT": PT1,
                                            "td": None, "Ysb": Ysb})
                            continue
                        if ch == 0 and h == 1:
                            continue
                        PT = pt_pool.tile([128, nt, 512], BF16, tag="pt")
                        t_old = pending.pop(0) if len(pending) == 2 else None
                        yp = emit_scores(qtc, h, PT, t_old,
                                         den_sg=5 if (ch, h) == (0, 2) else 1)
                        if t_old is not None:
                            finish(t_old, yp)
                        if h == 0 and ch + 1 < ng:
                            qtc_next = load_qtc(ch + 1)
                        if pending:
                            emit_tree(pending[-1])
                        pending.append(
                            {"ch": ch, "h": h, "PT": PT, "td": None, "Ysb": Ysb}
                        )
                emit_tree(pending[-1])
                while pending:
                    t_old = pending.pop(0)
                    emit_denominator(t_old)
                    yp = acc_ps.tile([128, 512], F32, tag="acc")
                    for i in range(nt):
                        nc.tensor.matmul(
                            yp[:],
                            lhsT=Vs[:, i, t_old["h"], :],
                            rhs=t_old["PT"][:, i, :],
                            start=(i == 0),
                            stop=(i == nt - 1),
                        )
                    finish(t_old, yp)
                while proj_units or proj_cur:
                    proj_step()

    nc.compile()
    return nc


def prep_inputs(x, cos, sin, w_qkv, w_proj, g_q, g_k, T_=T, b_count=B):
    """Host-side sharding: per-core input dicts."""
    x = np.asarray(x, dtype=np.float32)
    cos = np.asarray(cos, dtype=np.float32)[:T_]
    sin = np.asarray(sin, dtype=np.float32)[:T_]
    w_qkv = np.asarray(w_qkv, dtype=np.float32)
    w_proj = np.asarray(w_proj, dtype=np.float32)
    g_q = np.asarray(g_q, dtype=np.float32)
    g_k = np.asarray(g_k, dtype=np.float32)

    srcidx = np.concatenate([np.arange(64, 128), np.arange(0, 64)])
    sign = np.concatenate([-np.ones(64, np.float32), np.ones(64, np.float32)])
    scale_q = 1.0 / math.sqrt(D)
    # tables transposed to (d, t)
    cq_np = np.ascontiguousarray((cos * g_q[None, :] * scale_q).T)
    sq_np = np.ascontiguousarray(
        (sin * sign[None, :] * g_q[srcidx][None, :] * scale_q).T
    )
    ck_np = np.ascontiguousarray((cos * g_k[None, :]).T)
    sk_np = np.ascontiguousarray((sin * sign[None, :] * g_k[srcidx][None, :]).T)

    wq_r = w_qkv.reshape(3, H, D, C)
    wp_r = w_proj.reshape(C, H, D)
    ncl = C // 128

    in_maps = []
    for core in range(NCORES):
        b = core // 2
        hg = core % 2
        hsel = slice(hg * HL, (hg + 1) * HL)
        xt_np = np.ascontiguousarray(x[b % b_count][:T_].T).astype(NP_BF16)
        # [g, c_part, c_tile, d] with c = c_tile*128 + c_part
        kq = np.concatenate([wq_r[1, hsel], wq_r[0, hsel]], axis=0)  # [16, D, C]
        wqk_np = np.ascontiguousarray(
            kq.reshape(16, D, ncl, 128).transpose(0, 3, 2, 1)
        ).astype(NP_BF16)
        wv_np = np.ascontiguousarray(
            wq_r[2, hsel].reshape(HL * D, C).T
        ).astype(NP_BF16)  # [C, HL*D]
        wp_np = np.ascontiguousarray(
            wp_r[:, hsel, :].transpose(1, 2, 0)
        ).astype(NP_BF16)  # [HL, D, C]
        in_maps.append(
            {
                "xt": xt_np,
                "wqk": wqk_np,
                "wv": wv_np,
                "wp": wp_np,
                "cq": cq_np,
                "sq": sq_np,
                "ck": ck_np,
                "sk": sk_np,
            }
        )
    return in_maps


_nc_cache = {}


def _get_nc(T_=T):
    if T_ not in _nc_cache:
        _nc_cache[T_] = build(T_)
    return _nc_cache[T_]


def _install_trace_hook():
    """Register the axon NTFF profile hook (missing from this image's antenv)."""
    if "antenv.axon_hooks" in sys.modules:
        return
    try:
        from trn_agent_boot.trn_boot import _ntff_profile_via_ctypes

        hook = _ntff_profile_via_ctypes("/opt/axon/libaxon_pjrt.so")
        mod = types.ModuleType("antenv.axon_hooks")
        mod.get_axon_ntff_profile_hook = lambda: hook
        sys.modules["antenv.axon_hooks"] = mod
    except Exception:
        pass


def run(inputs, T_=T, trace=False, tmpdir=None):
    """Run the sharded kernel; returns (full output [B, T, C] fp32, results obj)."""
    nc = _get_nc(T_)
    in_maps = prep_inputs(**inputs, T_=T_)
    kwargs = {}
    if trace:
        _install_trace_hook()
        kwargs = dict(trace=True, tmpdir=tmpdir)
    res = run_bass_kernel_spmd(nc, in_maps, core_ids=list(range(NCORES)), **kwargs)
    outs = [res.results[i]["out"] for i in range(NCORES)]
    full = np.stack([outs[2 * b] + outs[2 * b + 1] for b in range(B)], axis=0).astype(
        np.float32
    )
    return full, res


def kernel(x, cos, sin, w_qkv, w_proj, g_q, g_k):
    full, _ = run(
        dict(x=x, cos=cos, sin=sin, w_qkv=w_qkv, w_proj=w_proj, g_q=g_q, g_k=g_k)
    )
    return full


# revision 47
# speedup vs baseline: 1.0129x; 1.0129x over previous
"""Fused multi-head attention block (QKV proj + per-head RMSNorm + RoPE +
softmax attention + output proj) on 8 Trainium2 NeuronCores.

Sharding: core c handles (batch b = c//2, head-group hg = c%2 of 8 heads).
Each core computes a partial output projection over its 8 heads; the host
sums the two partials per batch.

Optimizations over the straightforward schedule:
- q/k are produced directly transposed (d, t) by making the weight tile the
  matmul's stationary operand, which removes all PE transposes; RMS-norm
  sums become ones-matmuls over the partition dim, and rstd rows are
  broadcast down partitions by gpsimd.
- heads are processed in pairs so DVE/ACT ops are 1024 elements wide
  (per-op fixed overhead amortized).
- the softmax denominator is a wide strided DVE add-tree (16 -> 2) plus a
  2-step accumulating ones-matmul, instead of 8 ones-matmuls.
- reciprocals use the fast approximate DVE op (~18 bits, plenty for bf16).
- PV matmuls and output-projection matmuls are interleaved between score
  matmuls: exp on ACT is slower than the matmuls that fill each score
  bank, so the PE is given sp-independent work to chew on.

Self-contained: hardcodes B=4, T=2048, C=2048, H=16, D=128.
"""

import math
import sys
import types

import numpy as np
import ml_dtypes

import concourse.bass as bass
import concourse.bacc as bacc
import concourse.tile as tile
from concourse import mybir
from concourse.bass_utils import run_bass_kernel_spmd

BF16 = mybir.dt.bfloat16
F32 = mybir.dt.float32
NP_BF16 = ml_dtypes.bfloat16
AF = mybir.ActivationFunctionType
ALU = mybir.AluOpType

B, T, C, H, D = 4, 2048, 2048, 16, 128
HL = H // 2  # heads per core
EPS = 1e-6
NCORES = 8


def _bcast_mid(ap2d, n):
    """[P, F] AP -> [P, n, F] AP broadcast along a new middle dim (step 0)."""
    return bass.AP(
        tensor=ap2d.tensor,
        offset=ap2d.offset,
        ap=[ap2d.ap[0], [0, n], ap2d.ap[1]],
    )


def build(T_=T):
    """Build + compile the per-core Bass program (identical on all cores)."""
    nt = T_ // 128  # number of 128-row t-tiles
    ng = T_ // 512  # number of 512-col t-chunks
    ncl = C // 128  # contraction tiles over C

    nc = bacc.Bacc("TRN2", target_bir_lowering=False, debug=False, num_devices=NCORES)

    xt = nc.dram_tensor("xt", [C, T_], BF16, kind="ExternalInput")  # x[b].T
    # [g, c_part, c_tile, d]: g = 8 k-heads then 8 q-heads (k lands first);
    # per-g slice is 4KB/partition contiguous for fast DMA
    wqk = nc.dram_tensor("wqk", [16, 128, ncl, D], BF16, kind="ExternalInput")
    wv = nc.dram_tensor("wv", [C, HL * D], BF16, kind="ExternalInput")
    wp = nc.dram_tensor("wp", [HL, D, C], BF16, kind="ExternalInput")
    # rope tables transposed to (d, t); q tables carry g_q and 1/sqrt(D)
    cq = nc.dram_tensor("cq", [D, T_], F32, kind="ExternalInput")
    sq = nc.dram_tensor("sq", [D, T_], F32, kind="ExternalInput")
    ck = nc.dram_tensor("ck", [D, T_], F32, kind="ExternalInput")
    sk = nc.dram_tensor("sk", [D, T_], F32, kind="ExternalInput")
    out = nc.dram_tensor("out", [T_, C], F32, kind="ExternalOutput")

    with tile.TileContext(nc) as tc:
        with (
            tc.tile_pool(name="persist", bufs=1) as persist,
            tc.tile_pool(name="dram", bufs=1, space="DRAM") as dpool,
        ):
            ones_b = persist.tile([128, 1], BF16)
            nc.vector.memset(ones_b[:], 1.0)
            epsb = persist.tile([128, 1], F32)
            nc.vector.memset(epsb[:], EPS)

            KT = persist.tile([128, HL, T_], BF16)  # normalized k^T: (d, h, t)
            qt_dram = dpool.tile([HL, D, T_], BF16)  # q^T spill: (h, d, t)
            vs_dram = dpool.tile([128, nt, HL, D], BF16)  # v spill

            # ---------- phase 1: QKV proj + RMS norm + RoPE ----------
            with (
                tc.tile_pool(name="wqk_pool", bufs=1) as wqk_pool,
                tc.tile_pool(name="wv_pool", bufs=1) as wv_pool,
                tc.tile_pool(name="x_pool", bufs=2) as x_pool,
                tc.tile_pool(name="cs_pool", bufs=1) as cs_pool,
                tc.tile_pool(name="work", bufs=1) as work,
                tc.tile_pool(name="wk2", bufs=2) as wk2,
                tc.tile_pool(name="qk_ps", bufs=3, space="PSUM") as qk_ps,
                tc.tile_pool(name="s1_ps", bufs=2, space="PSUM") as s1_ps,
            ):
                def load_xp(ch):
                    xp = x_pool.tile([128, ncl, 512], BF16, tag="xp")
                    for cc in range(4):
                        nc.scalar.dma_start(
                            out=xp[:, cc * 4 : (cc + 1) * 4, :],
                            in_=xt[:].rearrange("(n p) t -> p n t", p=128)[
                                :, cc * 4 : (cc + 1) * 4, ch * 512 : (ch + 1) * 512
                            ],
                        )
                    return xp

                def load_tabs(ch):
                    tabs = {}
                    for nm, dram in (("cq", cq), ("sq", sq), ("ck", ck), ("sk", sk)):
                        tab = cs_pool.tile([128, 512], F32, tag=nm, name=nm)
                        nc.scalar.dma_start(
                            out=tab[:], in_=dram[:, ch * 512 : (ch + 1) * 512]
                        )
                        tabs[nm] = tab
                    return tabs

                def post_pair(ch, j, ps2, tabs):
                    """RMS-normalize + rope one head pair (k if j<4 else q)."""
                    is_k = j < 4
                    h0 = (j - (0 if is_k else 4)) * 2
                    # sum of squares per t (ones-matmul over the d partitions)
                    sqt2 = work.tile([128, 2, 512], BF16, tag="sqt2")
                    nc.scalar.activation(sqt2[:], ps2[:], AF.Square)
                    # rstd row = 1/sqrt(ss/D + eps), then broadcast down partitions
                    rb2 = wk2.tile([128, 2, 512], F32, tag="rb2")
                    for u in range(2):
                        ss = s1_ps.tile([1, 512], F32, tag="ss")
                        nc.tensor.matmul(
                            ss[:], lhsT=ones_b[:], rhs=sqt2[:, u, :],
                            start=True, stop=True,
                        )
                        sqb = work.tile([1, 512], F32, tag="sqb")
                        nc.scalar.activation(
                            sqb[:], ss[:], AF.Sqrt, bias=epsb[0:1, :], scale=1.0 / D
                        )
                        rr = work.tile([1, 512], F32, tag="rr")
                        nc.vector.reciprocal_approx_fast(rr[:], sqb[:])
                        nc.gpsimd.partition_broadcast(rb2[:, u, :], rr[:])
                    # rope (wide ops over the pair; sign folded into sin table)
                    ct, st = (tabs["ck"], tabs["sk"]) if is_k else (
                        tabs["cq"], tabs["sq"])
                    t3a = work.tile([128, 2, 512], F32, tag="t3a")
                    nc.vector.tensor_mul(t3a[:], ps2[:], _bcast_mid(ct[:], 2))
                    t3b = work.tile([128, 2, 512], F32, tag="t3b")
                    nc.vector.tensor_mul(
                        t3b[0:64, :, :], ps2[64:128, :, :],
                        _bcast_mid(st[0:64, :], 2),
                    )
                    nc.vector.tensor_mul(
                        t3b[64:128, :, :], ps2[0:64, :, :],
                        _bcast_mid(st[64:128, :], 2),
                    )
                    t3 = wk2.tile([128, 2, 512], F32, tag="t3")
                    nc.vector.tensor_add(t3[:], t3a[:], t3b[:])
                    if is_k:
                        nc.vector.tensor_mul(
                            KT[:, h0 : h0 + 2, ch * 512 : (ch + 1) * 512],
                            t3[:], rb2[:],
                        )
                    else:
                        qbf2 = work.tile([128, 2, 512], BF16, tag="qbf2")
                        nc.vector.tensor_mul(qbf2[:], t3[:], rb2[:])
                        nc.scalar.dma_start(
                            out=qt_dram[
                                h0 : h0 + 2, :, ch * 512 : (ch + 1) * 512
                            ].rearrange("h d t -> d h t"),
                            in_=qbf2[:],
                        )

                xp = load_xp(0)
                tabs = load_tabs(0)
                WQK = wqk_pool.tile([128, ncl, 16, D], BF16)
                for g in range(16):  # k heads 0-7 first, then q heads
                    nc.sync.dma_start(out=WQK[:, :, g, :], in_=wqk[g])
                WV = wv_pool.tile([128, ncl, HL * D], BF16)
                for cc in range(4):
                    nc.sync.dma_start(
                        out=WV[:, cc * 4 : (cc + 1) * 4, :],
                        in_=wv[:].rearrange("(n p) f -> p n f", p=128)[
                            :, cc * 4 : (cc + 1) * 4, :
                        ],
                    )
                for ch in range(ng):
                    # post-processing lags one item so the PE never waits on
                    # the ACT/DVE chain of the work it just produced.  The
                    # last chunk runs k, v, q so the sync queue's v spills
                    # finish early and phase 2's prefetches start sooner.
                    if ch + 1 < ng:
                        items = [("p", j) for j in range(8)]
                        items[4:4] = []
                        items += [("v", ts) for ts in range(4)]
                    else:
                        items = (
                            [("p", j) for j in range(4)]
                            + [("v", ts) for ts in range(4)]
                            + [("p", j) for j in range(4, 8)]
                        )
                    prev = None
                    for nitem, (kind, j) in enumerate(items):
                        ps2 = qk_ps.tile([128, 2, 512], F32, tag="qkps")
                        if kind == "p":
                            for u in range(2):
                                g = 2 * j + u if j < 4 else 8 + (j - 4) * 2 + u
                                for c in range(ncl):
                                    nc.tensor.matmul(
                                        ps2[:, u, :],
                                        lhsT=WQK[:, c, g, :],
                                        rhs=xp[:, c, :],
                                        start=(c == 0),
                                        stop=(c == ncl - 1),
                                    )
                        else:
                            for u in range(2):
                                for c in range(ncl):
                                    nc.tensor.matmul(
                                        ps2[:, u, :],
                                        lhsT=xp[:, c, j * 128 : (j + 1) * 128],
                                        rhs=WV[:, c, u * 512 : (u + 1) * 512],
                                        start=(c == 0),
                                        stop=(c == ncl - 1),
                                    )
                        if prev is not None:
                            pk, pj, pps = prev
                            if pk == "p":
                                post_pair(ch, pj, pps, tabs)
                            else:
                                vst = work.tile([128, 1024], BF16, tag="vst")
                                nc.scalar.copy(vst[:], pps[:])
                                nc.sync.dma_start(
                                    out=vs_dram[:, ch * 4 + pj, :, :],
                                    in_=vst[:].rearrange("p (h d) -> p h d", h=HL),
                                )
                        prev = (kind, j, ps2)
                        if nitem == 3 and ch + 1 < ng:
                            xp_next = load_xp(ch + 1)
                            tabs_next = load_tabs(ch + 1)
                    pk, pj, pps = prev
                    if pk == "p":
                        post_pair(ch, pj, pps, tabs)
                    else:
                        vst = work.tile([128, 1024], BF16, tag="vst")
                        nc.scalar.copy(vst[:], pps[:])
                        nc.sync.dma_start(
                            out=vs_dram[:, ch * 4 + pj, :, :],
                            in_=vst[:].rearrange("p (h d) -> p h d", h=HL),
                        )
                    if ch + 1 < ng:
                        xp = xp_next
                        tabs = tabs_next

            # ---------- phase 2: attention + output projection ----------
            with (
                tc.tile_pool(name="vs_pool", bufs=1) as vs_pool,
                tc.tile_pool(name="wp_pool", bufs=1) as wp_pool,
                tc.tile_pool(name="qt_pool", bufs=2) as qt_pool,
                tc.tile_pool(name="pt_pool", bufs=3) as pt_pool,
                tc.tile_pool(name="tree_pool", bufs=1) as tree_pool,
                tc.tile_pool(name="y_pool", bufs=2) as y_pool,
                tc.tile_pool(name="o_pool", bufs=3) as o_pool,
                tc.tile_pool(name="r_pool", bufs=2) as r_pool,
                tc.tile_pool(name="rb_pool", bufs=2) as rb_pool,
                tc.tile_pool(name="sp_ps", bufs=2, space="PSUM") as sp_ps,
                tc.tile_pool(name="acc_ps", bufs=4, space="PSUM") as acc_ps,
            ):
                def load_qtc(ch, queue=None):
                    qtc = qt_pool.tile([128, HL, 512], BF16, tag="qtc")
                    (queue or nc.scalar).dma_start(
                        out=qtc[:],
                        in_=qt_dram[:, :, ch * 512 : (ch + 1) * 512].rearrange(
                            "h d t -> d h t"
                        ),
                    )
                    return qtc

                # the sync queue is idle after the early weight loads, and this
                # load only waits on chunk 0's q spill: it runs mid-phase-1
                qtc_next = load_qtc(0, queue=nc.sync)
                Vs = vs_pool.tile([128, nt, HL, D], BF16)
                nc.sync.dma_start(out=Vs[:], in_=vs_dram[:])
                WPT = wp_pool.tile([128, HL, C], BF16)
                nc.sync.dma_start(out=WPT[:], in_=wp[:].rearrange("h d o -> d h o"))

                def emit_scores(qtc, h, PT, t_old, den_sg=1):
                    """S^T = K^T.T @ q^T; exp into PT (2-tile ACT ops).

                    The task from two iterations ago has its PV matmuls
                    interleaved between score matmuls: exp is slower than the
                    matmul that fills each score bank, so the PE needs
                    sp-independent work in between.
                    """
                    yp = None
                    if t_old is not None:
                        yp = acc_ps.tile([128, 512], F32, tag="acc")
                    for sg in range(nt // 2):
                        sp = sp_ps.tile([128, 2, 512], F32, tag="sp")
                        for u in range(2):
                            i = 2 * sg + u
                            nc.tensor.matmul(
                                sp[:, u, :],
                                lhsT=KT[:, h, i * 128 : (i + 1) * 128],
                                rhs=qtc[:, h, :],
                                start=True,
                                stop=True,
                            )
                        nc.scalar.activation(
                            PT[:, 2 * sg : 2 * sg + 2, :], sp[:], AF.Exp
                        )
                        if t_old is not None:
                            if sg == den_sg:
                                emit_denominator(t_old)
                            for u in range(2):
                                i = 2 * sg + u
                                nc.tensor.matmul(
                                    yp[:],
                                    lhsT=Vs[:, i, t_old["h"], :],
                                    rhs=t_old["PT"][:, i, :],
                                    start=(i == 0),
                                    stop=(i == nt - 1),
                                )
                        proj_step()
                    return yp

                def emit_tree(t):
                    """Pairwise add-tree 16 -> 2 as three wide strided DVE ops."""
                    PT = t["PT"]
                    ta = tree_pool.tile([128, 8, 512], BF16, tag="ta")
                    nc.vector.tensor_add(ta[:], PT[:, 0:16:2, :], PT[:, 1:16:2, :])
                    tb = tree_pool.tile([128, 4, 512], BF16, tag="tb")
                    nc.vector.tensor_add(tb[:], ta[:, 0:8:2, :], ta[:, 1:8:2, :])
                    tc2 = tree_pool.tile([128, 2, 512], BF16, tag="tc2")
                    nc.vector.tensor_add(tc2[:], tb[:, 0:4:2, :], tb[:, 1:4:2, :])
                    t["td"] = tc2

                def emit_denominator(t):
                    """Denominator ones-matmul + approx reciprocal + broadcast."""
                    ss = acc_ps.tile([1, 512], F32, tag="acc")
                    td = t["td"]
                    for j in range(2):
                        nc.tensor.matmul(ss[:], lhsT=ones_b[:], rhs=td[:, j, :],
                                         start=(j == 0), stop=(j == 1))
                    rinv = r_pool.tile([1, 512], F32, tag="rinv")
                    nc.vector.reciprocal_approx_fast(rinv[:], ss[:])
                    rbs = rb_pool.tile([128, 512], F32, tag="rbs")
                    nc.gpsimd.partition_broadcast(rbs[:], rinv[:])
                    t["rbs"] = rbs

                # output projection runs as a work queue: two matmuls are
                # drained per score group, absorbing the projection into the
                # exp-paced stretches where the PE otherwise has spare cycles
                proj_units = []  # (Ysb, trow, ot) per op-tile
                proj_cur = {}

                def proj_step():
                    if not proj_cur:
                        if not proj_units:
                            return
                        Ysb, trow, ot = proj_units.pop(0)
                        op = acc_ps.tile([128, 512], F32, tag="acc", name="op")
                        proj_cur.update(Ysb=Ysb, trow=trow, ot=ot, h=0, op=op)
                    Ysb, trow, ot = proj_cur["Ysb"], proj_cur["trow"], proj_cur["ot"]
                    op, h = proj_cur["op"], proj_cur["h"]
                    ts = trow % 4
                    for hh in (h, h + 1):
                        nc.tensor.matmul(
                            op[:],
                            lhsT=Ysb[:, hh, ts * 128 : (ts + 1) * 128],
                            rhs=WPT[:, hh, ot * 512 : (ot + 1) * 512],
                            start=(hh == 0),
                            stop=(hh == HL - 1),
                        )
                    proj_cur["h"] = h + 2
                    if proj_cur["h"] == HL:
                        ost = o_pool.tile([128, 512], F32, tag="ost")
                        nc.vector.tensor_copy(ost[:], op[:])
                        nc.sync.dma_start(
                            out=out[:].rearrange("(n p) o -> n p o", p=128)[
                                trow
                            ][:, ot * 512 : (ot + 1) * 512],
                            in_=ost[:],
                        )
                        proj_cur.clear()

                def emit_proj(ch, Ysb):
                    """Queue the output projection for one finished chunk."""
                    for ts in range(4):
                        for ot in range(C // 512):
                            proj_units.append((Ysb, ch * 4 + ts, ot))

                pending = []  # task dicts; PV interleaves two iterations later

                def finish(t, yp):
                    nc.vector.tensor_mul(t["Ysb"][:, t["h"], :], yp[:], t["rbs"][:])
                    if t["h"] == HL - 1:
                        emit_proj(t["ch"], t["Ysb"])

                for ch in range(ng):
                    qtc = qtc_next
                    Ysb = y_pool.tile([128, HL, 512], BF16, tag="y")
                    for h in range(HL):
                        PT = pt_pool.tile([128, nt, 512], BF16, tag="pt")
                        t_old = pending.pop(0) if len(pending) == 2 else None
                        yp = emit_scores(qtc, h, PT, t_old)
                        if t_old is not None:
                            finish(t_old, yp)
                        if h == 0 and ch + 1 < ng:
                            qtc_next = load_qtc(ch + 1)
                        if pending:
                            emit_tree(pending[-1])
                        pending.append(
                            {"ch": ch, "h": h, "PT": PT, "td": None, "Ysb": Ysb}
                        )
                emit_tree(pending[-1])
                while pending:
                    t_old = pending.pop(0)
                    emit_denominator(t_old)
                    yp = acc_ps.tile([128, 512], F32, tag="acc")
                    for i in range(nt):
                        nc.tensor.matmul(
                            yp[:],
                            lhsT=Vs[:, i, t_old["h"], :],
                            rhs=t_old["PT"][:, i, :],
                            start=(i == 0),
                            stop=(i == nt - 1),
                        )
                    finish(t_old, yp)
                while proj_units or proj_cur:
                    proj_step()

    nc.compile()
    return nc


def prep_inputs(x, cos, sin, w_qkv, w_proj, g_q, g_k, T_=T, b_count=B):
    """Host-side sharding: per-core input dicts."""
    x = np.asarray(x, dtype=np.float32)
    cos = np.asarray(cos, dtype=np.float32)[:T_]
    sin = np.asarray(sin, dtype=np.float32)[:T_]
    w_qkv = np.asarray(w_qkv, dtype=np.float32)
    w_proj = np.asarray(w_proj, dtype=np.float32)
    g_q = np.asarray(g_q, dtype=np.float32)
    g_k = np.asarray(g_k, dtype=np.float32)

    srcidx = np.concatenate([np.arange(64, 128), np.arange(0, 64)])
    sign = np.concatenate([-np.ones(64, np.float32), np.ones(64, np.float32)])
    scale_q = 1.0 / math.sqrt(D)
    # tables transposed to (d, t)
    cq_np = np.ascontiguousarray((cos * g_q[None, :] * scale_q).T)
    sq_np = np.ascontiguousarray(
        (sin * sign[None, :] * g_q[srcidx][None, :] * scale_q).T
    )
    ck_np = np.ascontiguousarray((cos * g_k[None, :]).T)
    sk_np = np.ascontiguousarray((sin * sign[None, :] * g_k[srcidx][None, :]).T)

    wq_r = w_qkv.reshape(3, H, D, C)
    wp_r = w_proj.reshape(C, H, D)
    ncl = C // 128

    in_maps = []
    for core in range(NCORES):
        b = core // 2
        hg = core % 2
        hsel = slice(hg * HL, (hg + 1) * HL)
        xt_np = np.ascontiguousarray(x[b % b_count][:T_].T).astype(NP_BF16)
        # [g, c_part, c_tile, d] with c = c_tile*128 + c_part
        kq = np.concatenate([wq_r[1, hsel], wq_r[0, hsel]], axis=0)  # [16, D, C]
        wqk_np = np.ascontiguousarray(
            kq.reshape(16, D, ncl, 128).transpose(0, 3, 2, 1)
        ).astype(NP_BF16)
        wv_np = np.ascontiguousarray(
            wq_r[2, hsel].reshape(HL * D, C).T
        ).astype(NP_BF16)  # [C, HL*D]
        wp_np = np.ascontiguousarray(
            wp_r[:, hsel, :].transpose(1, 2, 0)
        ).astype(NP_BF16)  # [HL, D, C]
        in_maps.append(
            {
                "xt": xt_np,
                "wqk": wqk_np,
                "wv": wv_np,
                "wp": wp_np,
                "cq": cq_np,
                "sq": sq_np,
                "ck": ck_np,
                "sk": sk_np,
            }
        )
    return in_maps


_nc_cache = {}


def _get_nc(T_=T):
    if T_ not in _nc_cache:
        _nc_cache[T_] = build(T_)
    return _nc_cache[T_]


def _install_trace_hook():
    """Register the axon NTFF profile hook (missing from this image's antenv)."""
    if "antenv.axon_hooks" in sys.modules:
        return
    try:
        from trn_agent_boot.trn_boot import _ntff_profile_via_ctypes

        hook = _ntff_profile_via_ctypes("/opt/axon/libaxon_pjrt.so")
        mod = types.ModuleType("antenv.axon_hooks")
        mod.get_axon_ntff_profile_hook = lambda: hook
        sys.modules["antenv.axon_hooks"] = mod
    except Exception:
        pass


def run(inputs, T_=T, trace=False, tmpdir=None):
    """Run the sharded kernel; returns (full output [B, T, C] fp32, results obj)."""
    nc = _get_nc(T_)
    in_maps = prep_inputs(**inputs, T_=T_)
    kwargs = {}
    if trace:
        _install_trace_hook()
        kwargs = dict(trace=True, tmpdir=tmpdir)
    res = run_bass_kernel_spmd(nc, in_maps, core_ids=list(range(NCORES)), **kwargs)
    outs = [res.results[i]["out"] for i in range(NCORES)]
    full = np.stack([outs[2 * b] + outs[2 * b + 1] for b in range(B)], axis=0).astype(
        np.float32
    )
    return full, res


def kernel(x, cos, sin, w_qkv, w_proj, g_q, g_k):
    full, _ = run(
        dict(x=x, cos=cos, sin=sin, w_qkv=w_qkv, w_proj=w_proj, g_q=g_q, g_k=g_k)
    )
    return full
